# revision 1
# baseline (speedup 1.0000x reference)
"""Trainium2 Bass kernel for LogisticRegressionRBF.

Computes sigmoid(exp(-||x_i - c_j||^2) @ w + b) for x [K, M], c [N, M],
w [N], b [1] with K = N = 8192, M = 64, sharded data-parallel over rows
of x across 8 NeuronCores.

Algorithm (per core, KS = K/8 = 1024 rows):
  - Host folds everything into one bf16 matmul via feature augmentation
    (67 features): with A = 2*log2(e)*2^23 and B = 127*2^23,
        xhat_k = [x_k, -||x_k||^2/2, 1, 1]
        chat_n = [A*c_n, A, A*(-||c_n||^2 + ln|w_n|)/2, B]
    so the PE produces P_kn = A*R_kn + B in PSUM, where
    2*R_kn = -||x_k - c_n||^2 + ln|w_n| and exp(2R) = |w_n| * phi_kn.
  - Basis columns are pre-sorted by sign(w) on the host (the n-sum is
    permutation invariant), so sum_n w_n phi_kn = S_pos - S_neg with
    each S a plain sum over a contiguous column range.
  - exp + row-sum of each 1024-column PSUM chunk runs on ONE of two
    engines, statically load-balanced ~59/41 so both stay saturated:
      * ACT: Exp(P*EXP_SCALE + EXP_BIAS) in place, with accum_out
        emitting the per-row partial sums for free (fused reduce);
      * DVE: Schraudolph bits — int32(max(P, 2^23)) IS the exp2 bit
        pattern; two pairwise folds on the otherwise-idle GPSIMD shrink
        the DVE bitcast-reduce to chunk/4.
  - A tiny DVE combine applies the +/- signs and adds b; sigmoid is
    0.5*tanh(z/2) + 0.5, flushed in two halves so the first output DMA
    overlaps the remaining chunk work. A 1-col warm-up matmul at t~0
    starts the PE p-state ramp during the DMA lead-in, and the first cT
    chunk is DMA'd before xT so the first real matmul starts earlier.
"""

import os
import sys
from contextlib import ExitStack

import numpy as np

try:
    import concourse.bass as bass  # noqa: F401
except ImportError:  # fresh grading dir: framework lives on these paths
    for _p in (
        "/root/.axon_site/_ro/trn_rl_repo",
        "/root/.axon_site/_ro/pypackages",
        "/opt/trn_rl_repo",
        "/opt/pypackages",
    ):
        if os.path.isdir(_p) and _p not in sys.path:
            sys.path.append(_p)
    import concourse.bass as bass  # noqa: F401

import concourse.tile as tile
from concourse import bacc, mybir
from concourse.bass_utils import run_bass_kernel_spmd

F32 = mybir.dt.float32
AF = mybir.ActivationFunctionType
ALU = mybir.AluOpType

N_CORES = 8
CHUNK = 1024  # exp-chunk granularity (PSUM tile columns)
PSUM_BUFS = 4
NT = 512      # matmul moving-operand free dim: 1 PSUM bank (fp32 max)

# Schraudolph exp2 bit-trick, folded into the matmul:
# basis features are pre-scaled by A = 2*log2(e)*2^23 and B = 127*2^23 is
# added via an extra augmented feature row, so PSUM holds P = A*R + B
# directly (where 2R = -||x-c||^2 + ln|w|, always << 0).
#   DVE path:  exp(2R) ~= bitcast_f32(int32(max(P, 2^23)))   (~3% rel err —
#     irrelevant here: every phi is ~1e-17 against an output of 0.5)
#   ACT path:  exp(2R) = Exp(P * EXP_SCALE + EXP_BIAS)  (exact)
# The lower clamp keeps the biased exponent >= 1 (no denormals, no
# negative-int garbage); P never overflows upward since 2R < ln(max|w|).
import ml_dtypes
EXP_A = float(np.float32(ml_dtypes.bfloat16(
    2.0 * 1.4426950408889634 * (1 << 23))))  # bf16-exact, used on host & chip
EXP_B = float(127 * (1 << 23))               # bf16-exact
EXP_CLAMP = float(1 << 23)                   # lower clamp on P
EXP_SCALE = float(np.float32(2.0 / EXP_A))
EXP_BIAS = float(np.float32(-EXP_B * (2.0 / EXP_A)))
# chunks with (global_chunk_idx % DVE_MOD) in DVE_PICK run on the DVE
# (~41% DVE / ~59% ACT — balances both engines; the spread was tuned
# against the cost-model timeline, with the tail biased toward ACT so
# the DVE backlog doesn't starve ACT at the end of the schedule)
DVE_MOD = 32
DVE_PICK = frozenset({1, 3, 6, 8, 11, 13, 16, 18, 19, 21, 23, 26, 28})


def set_config(chunk=None, psum_bufs=None, dve_frac=None):
    """Tune chunk size / psum buffering / DVE share (for config sweeps)."""
    global CHUNK, PSUM_BUFS, DVE_PICK
    if chunk is not None:
        CHUNK = chunk
    if psum_bufs is not None:
        PSUM_BUFS = psum_bufs
    if dve_frac is not None:
        count = max(0, min(DVE_MOD, round(dve_frac * DVE_MOD)))
        picks = set()
        i = 0
        while len(picks) < count:
            picks.add((1 + int(round(i * DVE_MOD / count))) % DVE_MOD)
            i += 1
        DVE_PICK = frozenset(picks)

LAST_RESULT = None  # BassKernelResults of the most recent run (for test.py)


def _plan_ranges(p_pos: int, n: int, chunk: int):
    """Sign-pure (lo, hi, sign) ranges per column chunk (chunk-relative)."""
    ranges = []
    for c0 in range(0, n, chunk):
        c1 = c0 + chunk
        if p_pos <= c0:
            ent = [(0, chunk, -1.0)]
        elif p_pos >= c1:
            ent = [(0, chunk, 1.0)]
        else:
            ent = [(0, p_pos - c0, 1.0), (p_pos - c0, chunk, -1.0)]
        ranges.append(ent)
    return ranges


def _build(nc, ks: int, n: int, c_dim: int, ranges, ncols: int,
           chunk: int, nt: int):
    BF16 = mybir.dt.bfloat16
    xT = nc.dram_tensor("xT", [c_dim, ks], BF16, kind="ExternalInput").ap()
    cT = nc.dram_tensor("cT", [c_dim, n], BF16, kind="ExternalInput").ap()
    sgn = nc.dram_tensor("sgn", [128, ncols], F32, kind="ExternalInput").ap()
    brep = nc.dram_tensor("brep", [128, 1], F32, kind="ExternalInput").ap()
    out = nc.dram_tensor("out", [ks, 1], F32, kind="ExternalOutput").ap()

    n_chunks = n // chunk
    n_ktiles = ks // 128

    with tile.TileContext(nc) as tc, ExitStack() as ctx:
        consts = ctx.enter_context(tc.tile_pool(name="consts", bufs=1))
        psum_pool = ctx.enter_context(
            tc.tile_pool(name="psum", bufs=PSUM_BUFS, space="PSUM"))
        spool = ctx.enter_context(tc.tile_pool(name="scols", bufs=3))
        small = ctx.enter_context(tc.tile_pool(name="small", bufs=4))
        dvework = ctx.enter_context(tc.tile_pool(name="dvework", bufs=3))

        # the first cT chunk + xT gate the first matmul — issue them first
        cT_sb = consts.tile([c_dim, n], BF16, tag="cT_sb")
        nc.sync.dma_start(cT_sb[:, :chunk], cT[:, :chunk])
        xT_sb = consts.tile([c_dim, ks], BF16, tag="xT_sb")
        nc.sync.dma_start(xT_sb[:], xT[:])
        # 2048-wide loads: halves the ~625ns/DMA HWDGE prep serialization
        for lo in range(chunk, n, 2 * chunk):
            hi = min(n, lo + 2 * chunk)
            nc.sync.dma_start(cT_sb[:, lo:hi], cT[:, lo:hi])
        sgn_sb = consts.tile([128, ncols], F32, tag="sgn_sb")
        nc.sync.dma_start(sgn_sb[:], sgn[:])
        b_sb = consts.tile([128, 1], F32, tag="b_sb")
        nc.sync.dma_start(b_sb[:], brep[:])
        ebias_sb = consts.tile([128, 1], F32, tag="ebias_sb")
        nc.vector.memset(ebias_sb[:], EXP_BIAS)
        warm_ps = psum_pool.tile([128, CHUNK], F32, tag="ps")
        nc.tensor.matmul(warm_ps[:1, :1], ebias_sb[:], ebias_sb[:],
                         start=True, stop=True)

        I32 = mybir.dt.int32
        z_all = consts.tile([128, n_ktiles], F32, tag="z_all")
        res_all = consts.tile([128, n_ktiles], F32, tag="res_all")
        pending = []  # deferred DVE reduces: (src_ap, col_ap) — issued one
                      # chunk late so they don't head-of-line-block the DVE
                      # FIFO while the Pool folds run (depth 1 beat 2 by a
                      # hair in the cost-model timeline)

        def flush_pending(upto):
            while len(pending) > upto:
                src, dst = pending.pop(0)
                nc.vector.reduce_sum(dst, src, axis=mybir.AxisListType.X)

        for kt in range(n_ktiles):
            lhsT = xT_sb[:, kt * 128:(kt + 1) * 128]
            scols = spool.tile([128, ncols], F32, tag="scols")
            col = 0
            for ch in range(n_chunks):
                ps = psum_pool.tile([128, chunk], F32, tag="ps")
                for q in range(chunk // nt):
                    nc.tensor.matmul(
                        ps[:, q * nt:(q + 1) * nt],
                        lhsT,
                        cT_sb[:, ch * chunk + q * nt: ch * chunk + (q + 1) * nt],
                        start=True, stop=True)
                gidx = kt * n_chunks + ch
                if (gidx % DVE_MOD) in DVE_PICK:
                    # DVE exp path (Schraudolph), frees the ACT engine
                    t2 = dvework.tile([128, chunk], I32, tag="t2")
                    nc.vector.tensor_scalar_max(t2[:], ps[:], EXP_CLAMP)
                    t2f = t2[:].bitcast(F32)
                    if len(ranges[ch]) == 1 and chunk % 4 == 0:
                        # sign-pure chunk: two pairwise folds on the idle
                        # Pool engine shrink the DVE reduce to chunk/4
                        h, q4 = chunk // 2, chunk // 4
                        f1 = dvework.tile([128, h], F32, tag="f1")
                        nc.gpsimd.tensor_add(f1[:], t2f[:, :h], t2f[:, h:])
                        f2 = dvework.tile([128, q4], F32, tag="f2")
                        nc.gpsimd.tensor_add(f2[:], f1[:, :q4], f1[:, q4:])
                        pending.append((f2[:], scols[:, col:col + 1]))
                        col += 1
                    else:
                        for (lo, hi, _s) in ranges[ch]:
                            pending.append(
                                (t2f[:, lo:hi], scols[:, col:col + 1]))
                            col += 1
                    flush_pending(1)
                else:
                    for (lo, hi, _s) in ranges[ch]:
                        nc.scalar.activation(
                            ps[:, lo:hi], ps[:, lo:hi], AF.Exp,
                            scale=EXP_SCALE, bias=ebias_sb[:],
                            accum_out=scols[:, col:col + 1])
                        col += 1
            flush_pending(0)
            assert col == ncols
            tmp = small.tile([128, ncols], F32, tag="tmp")
            nc.vector.tensor_mul(tmp[:], scols[:], sgn_sb[:])
            zs = small.tile([128, 1], F32, tag="zs")
            nc.vector.reduce_sum(zs[:], tmp[:], axis=mybir.AxisListType.X)
            nc.vector.tensor_scalar_add(z_all[:, kt:kt + 1], zs[:], b_sb[:])
        # sigmoid tail in two halves: the first half's DMA overlaps the
        # second half's chunk work
        th_all = consts.tile([128, n_ktiles], F32, tag="th_all")
        out_view = out.rearrange("(a b) c -> b (a c)", b=128)
        hk = n_ktiles // 2
        for (lo2, hi2) in ((0, hk), (hk, n_ktiles)):
            nc.scalar.activation(th_all[:, lo2:hi2], z_all[:, lo2:hi2],
                                 AF.Tanh, scale=0.5)
            nc.vector.tensor_scalar(res_all[:, lo2:hi2], th_all[:, lo2:hi2],
                                    0.5, 0.5, ALU.mult, ALU.add)
            nc.sync.dma_start(out_view[:, lo2:hi2], res_all[:, lo2:hi2])


def _prep(x, x_basis, w, b):
    """Host-side: sign-sort basis columns, build augmented transposed mats."""
    x = np.asarray(x, np.float32)
    xb = np.asarray(x_basis, np.float32)
    w = np.asarray(w, np.float32)
    b = np.asarray(b, np.float32)
    k, m = x.shape
    n = xb.shape[0]

    order = np.argsort(w < 0, kind="stable")  # w >= 0 first
    cs = xb[order]
    ws = w[order]
    p_pos = int((w >= 0).sum())
    with np.errstate(divide="ignore"):
        lw = np.where(ws == 0.0, -1e30, np.log(np.abs(ws, dtype=np.float64)))
    xsq = np.einsum("km,km->k", x, x, dtype=np.float64)
    csq = np.einsum("nm,nm->n", cs, cs, dtype=np.float64)

    xT = np.empty((m + 3, k), np.float32)
    xT[:m] = x.T
    xT[m] = -xsq / 2.0
    xT[m + 1] = 1.0
    xT[m + 2] = 1.0

    cT = np.empty((m + 3, n), np.float32)
    cT[:m] = cs.T * EXP_A
    cT[m] = EXP_A
    cT[m + 1] = EXP_A * (-csq + lw) / 2.0
    cT[m + 2] = EXP_B
    return xT, cT, p_pos, b


def host_setup(x, x_basis, w, b):
    """Everything host-side: returns (build_args, in_maps, dims)."""
    import ml_dtypes

    k, m = x.shape
    n = x_basis.shape[0]
    ks = k // N_CORES
    c_dim = m + 3

    xT, cT, p_pos, b32 = _prep(x, x_basis, w, b)
    ranges = _plan_ranges(p_pos, n, CHUNK)
    signs = [s for ent in ranges for (_lo, _hi, s) in ent]
    ncols = len(signs)
    sgn = np.tile(np.asarray(signs, np.float32)[None, :], (128, 1))
    brep = np.full((128, 1), float(b32[0]), np.float32)

    xT16 = xT.astype(ml_dtypes.bfloat16)
    cT16 = np.ascontiguousarray(cT.astype(ml_dtypes.bfloat16))
    in_maps = [
        {
            "xT": np.ascontiguousarray(xT16[:, cid * ks:(cid + 1) * ks]),
            "cT": cT16,
            "sgn": sgn,
            "brep": brep,
        }
        for cid in range(N_CORES)
    ]
    build_args = dict(ks=ks, n=n, c_dim=c_dim, ranges=ranges, ncols=ncols,
                      chunk=CHUNK, nt=NT)
    return build_args, in_maps


def kernel(x, x_basis, w, b):
    global LAST_RESULT
    build_args, in_maps = host_setup(x, x_basis, w, b)
    nc = bacc.Bacc("TRN2", target_bir_lowering=False, debug=False,
                   num_devices=N_CORES)
    _build(nc, **build_args)
    nc.compile()
    r = run_bass_kernel_spmd(
        nc, in_maps, list(range(N_CORES)),
        trace=bool(os.environ.get("BASS_KERNEL_TRACE")))
    LAST_RESULT = r
    return np.concatenate([r.results[i]["out"] for i in range(N_CORES)], 0)



# revision 12
# speedup vs baseline: 1.1890x; 1.1890x over previous
"""Trainium2 Bass kernel for LogisticRegressionRBF.

Computes sigmoid(exp(-||x_i - c_j||^2) @ w + b) for x [K, M], c [N, M],
w [N], b [1] with K = N = 8192, M = 64, sharded data-parallel over rows
of x across 8 NeuronCores (KS = 1024 rows per core).

v2 architecture — transposed tiles + fp8 DoubleRow matmuls:
  - All feature matrices are fp8e4m3, with the e4m3 Schraudolph constants
    folded in: the PE produces P[n, x] = A8*R + B8 in PSUM where R =
    -||x - c||^2, A8 = 8*log2(e), B8 = 56 — so int8(max(P, 0)) IS the
    e4m3 bit pattern of exp(R). The 68-dim augmented contraction is split
    into 2 slices of 34 and both main and reduce matmuls run in fp8
    DoubleRow mode (0.5 PE cycles/row, 2x bf16 throughput).
  - Layout is transposed vs v1: each PSUM tile is phi^T [128 n-rows, 512
    x-cols], so the weighted n-reduction is itself a (DoubleRow) matmul:
    lhsT = a [128, 2, 128] view of a packed w table (col 0 = the real
    w pair, cols 1..127 garbage that lands in never-read PSUM partitions
    — the dual-fp8 ldweights ISA check requires 2 contiguous 128-col
    slices), rhs = the exp'd phi pair, accumulated into one PSUM bank
    per x-block across all 32 n-tile pairs.  No sign-sorting, no ln|w|
    folding, no on-engine reductions at all.
  - The exp itself is the only per-element work: PSUM chunks of 3 n-tiles
    [128, 1536] go to ACT (Exp activation, fp8 out) or DVE (Schraudolph
    tensor_scalar_max f32 -> int8, bitcast fp8), statically load-balanced.
  - Tail per x-block: sigmoid(z) = 0.5*tanh((z/256 + b)/2) + 0.5 on the
    [1, 512] z row; block 0's tail overlaps block 1's compute.
"""

import os
import sys
from contextlib import ExitStack

import numpy as np

try:
    import concourse.bass as bass  # noqa: F401
except ImportError:  # fresh grading dir: framework lives on these paths
    for _p in (
        "/root/.axon_site/_ro/trn_rl_repo",
        "/root/.axon_site/_ro/pypackages",
        "/opt/trn_rl_repo",
        "/opt/pypackages",
    ):
        if os.path.isdir(_p) and _p not in sys.path:
            sys.path.append(_p)
    import concourse.bass as bass  # noqa: F401

import ml_dtypes
import concourse.tile as tile
from concourse import bacc, mybir
from concourse.bass_utils import run_bass_kernel_spmd

F32 = mybir.dt.float32
F8 = mybir.dt.float8e4
I8 = mybir.dt.int8
AF = mybir.ActivationFunctionType
PM = mybir.MatmulPerfMode
NPF8 = ml_dtypes.float8_e4m3

N_CORES = 8
NT = 512            # x-block width (PSUM bank / matmul moving free dim)
PHI_BUFS = 8        # phi pair-tile buffering depth
W_SCALE = 256.0     # w prescale so fp8 w doesn't underflow (undone in tail)

# e4m3 Schraudolph: bits(exp(R)) = A8*R + B8 for R <= 0
A8 = 8.0 * 1.4426950408889634   # 8*log2(e)
B8 = 56.0                        # 7 (bias) * 8
EXP_SCALE = float(np.float32(1.0 / A8))       # ACT path: exp(P*s + c)
EXP_BIAS = float(np.float32(-B8 / A8))
NORM_CLAMP = 150.0  # host clamp on ||.||^2 so A8/8*norm stays in fp8 range

# chunk engine assignment, chunk = pair = [128, 1024] f32 in PSUM:
#   ACT: Exp activation straight from PSUM -> fp8 SBUF
#   DVE: tensor_scalar_max f32 -> int8 straight from PSUM
#   POOL: DMA stages the PSUM chunk to SBUF f32, then GPSIMD does the
#         Schraudolph max from SBUF (GPSIMD cannot touch PSUM); the DMA
#         engines are otherwise idle, so this is a free third drain
# greedy-balanced statically below using per-chunk cost estimates (ns)
ACT_COST = 1223.0
DVE_COST = 1192.0

LAST_RESULT = None  # BassKernelResults of the most recent run (for test.py)


def _chunk_plan(n_chunks_total):
    """Greedy A/D balance (31A/33D, mostly alternating) — measured best:
    strict-er patterns and ACT-heavier splits both sim slower; the split
    must fold in ACT's fixed work (act-table load, warm-up, tanh tails)."""
    busy = {"A": 0.0, "D": 0.0}
    cost = {"A": ACT_COST, "D": DVE_COST}
    assign = []
    for _ in range(n_chunks_total):
        k = min(busy, key=lambda e: busy[e] + cost[e])
        assign.append(k)
        busy[k] += cost[k]
    return assign


def _build(nc, ks: int, n: int, b_half: float):
    n_tiles = n // 128          # 64
    n_pairs = n_tiles // 2      # 32
    n_blocks = ks // NT         # 2
    assign = _chunk_plan(n_pairs * n_blocks)

    xT = nc.dram_tensor("xT", [34, n_blocks, 2, NT], F8,
                        kind="ExternalInput").ap()
    cT = nc.dram_tensor("cT", [34, n_tiles, 2, 128], F8,
                        kind="ExternalInput").ap()
    wq = nc.dram_tensor("wq", [128, 288], F8, kind="ExternalInput").ap()
    out = nc.dram_tensor("out", [ks, 1], F32, kind="ExternalOutput").ap()

    with tile.TileContext(nc) as tc, ExitStack() as ctx:
        consts = ctx.enter_context(tc.tile_pool(name="consts", bufs=1))
        psum_pool = ctx.enter_context(
            tc.tile_pool(name="psum", bufs=3, space="PSUM"))
        zpool = ctx.enter_context(
            tc.tile_pool(name="zpool", bufs=2, space="PSUM"))

        # warm-up: f32 1x1 matmul at t~0 pins pe_busy_start to ~0 so all
        # post-3us matmuls run at the full 2.4 GHz p-state; an early Exp
        # activation eats the ACT table load during the DMA lead-in
        warm = consts.tile([128, 1], F32, tag="warm")
        nc.vector.memset(warm[:], EXP_BIAS)
        wps = zpool.tile([128, NT], F32, tag="zb")
        nc.tensor.matmul(wps[:1, :1], warm[:], warm[:], start=True, stop=True)
        warm8 = consts.tile([128, 1], F8, tag="warm8")
        nc.scalar.activation(warm8[:], warm[:], AF.Exp, scale=1.0)

        # DMA lead-in: xT + the first cT tiles gate the first matmul;
        # later cT pieces land well before their chunks drain
        xT_sb = consts.tile([34, n_blocks, 2, NT], F8, tag="xT_sb")
        nc.sync.dma_start(xT_sb[:], xT[:])
        cT_sb = consts.tile([34, n_tiles, 2, 128], F8, tag="cT_sb")
        nc.sync.dma_start(cT_sb[:, :8], cT[:, :8])
        wq_sb = consts.tile([128, 288], F8, tag="wq_sb")
        nc.sync.dma_start(wq_sb[:], wq[:])
        nc.sync.dma_start(cT_sb[:, 8:24], cT[:, 8:24])
        nc.sync.dma_start(cT_sb[:, 24:], cT[:, 24:])

        ebias = consts.tile([128, 1], F32, tag="ebias")
        nc.vector.memset(ebias[:], EXP_BIAS)
        bhalf = consts.tile([1, 1], F32, tag="bhalf")
        nc.vector.memset(bhalf[:], b_half)

        phi_pool = ctx.enter_context(
            tc.tile_pool(name="phi_pool", bufs=PHI_BUFS))

        th = consts.tile([1, ks], F32, tag="th")
        res = consts.tile([1, ks], F32, tag="res")
        out_view = out.rearrange("(a b) c -> b (a c)", b=1)

        wq_base = wq_sb[:]
        wq_pdim = list(wq_base.ap)[0]

        def w_pair_view(j):
            # [[p,128],[128,2],[1,128]] at offset j: slice i col 0 reads
            # wq[p, j + 128 i] = w tile (2j + i); cols 1.. read garbage that
            # lands in never-read PSUM partitions 1..127
            return bass.AP(
                tensor=wq_base.tensor,
                offset=wq_base.offset + j,
                ap=[list(wq_pdim), [128, 2], [1, 128]],
            )

        zb = [zpool.tile([128, NT], F32, tag="zb", name=f"zb{i}")
              for i in range(n_blocks)]

        def emit_reduce(blk, j, phi_t):
            nc.tensor.matmul(
                zb[blk][:],
                w_pair_view(j),
                phi_t[:].rearrange("p (a b) -> p a b", a=2),
                start=(j == 0), stop=(j == n_pairs - 1),
                perf_mode=PM.DoubleRow)

        def emit_tail(blk):
            # z = row 0 of zb; sigmoid(z/W_SCALE + b) via same-table tanh
            o0 = blk * NT
            nc.scalar.activation(th[:, o0:o0 + NT], zb[blk][0:1, :],
                                 AF.Tanh, scale=float(np.float32(
                                     1.0 / (2.0 * W_SCALE))),
                                 bias=bhalf[:])
            nc.vector.tensor_scalar(res[:, o0:o0 + NT], th[:, o0:o0 + NT],
                                    0.5, 0.5, mybir.AluOpType.mult,
                                    mybir.AluOpType.add)
            nc.sync.dma_start(out_view[:, o0:o0 + NT], res[:, o0:o0 + NT])

        ci = 0
        for blk in range(n_blocks):
            xv = xT_sb[:, blk]          # [34, 2, NT]
            phi_tiles = {}
            for c in range(n_pairs):    # chunk c == n-tile pair c
                if blk > 0 and c == 6:
                    # previous block's tail, emitted after this block's
                    # pipeline has refilled so the tanh (queued on ACT)
                    # never stalls the exp stream at the block transition
                    emit_tail(blk - 1)
                # reduces are delayed 2 chunks so they are dep-satisfied at
                # dispatch and never head-block the in-order PE queue
                if c >= 2:
                    emit_reduce(blk, c - 2, phi_tiles.pop(c - 2))
                ps = psum_pool.tile([128, 2 * NT], F32, tag="ps")
                for i in range(2):
                    nc.tensor.matmul(
                        ps[:, i * NT:(i + 1) * NT],
                        cT_sb[:, 2 * c + i],    # [34, 2, 128]
                        xv,
                        start=True, stop=True,
                        perf_mode=PM.DoubleRow)
                phi_t = phi_pool.tile([128, 2 * NT], F8, tag="phi",
                                      name=f"phi_{blk}_{c}")
                phi_tiles[c] = phi_t
                if assign[ci] == "A":
                    nc.scalar.activation(
                        phi_t[:], ps[:], AF.Exp,
                        scale=EXP_SCALE, bias=ebias[:])
                else:
                    nc.vector.tensor_scalar_max(
                        phi_t[:].bitcast(I8), ps[:], 0.0)
                ci += 1
            for c in (n_pairs - 2, n_pairs - 1):
                emit_reduce(blk, c, phi_tiles.pop(c))
        emit_tail(n_blocks - 1)


def host_setup(x, x_basis, w, b):
    """Host-side fp8 feature construction. Returns (build_args, in_maps)."""
    x = np.asarray(x, np.float64)
    c = np.asarray(x_basis, np.float64)
    w64 = np.asarray(w, np.float64)
    b64 = np.asarray(b, np.float64)
    k, m = x.shape
    n = c.shape[0]
    ks = k // N_CORES
    n_blocks = ks // NT

    s_cross = np.sqrt(2.0 * A8)
    s_norm = A8 / 8.0

    def feats(mat):
        # [68, rows]: 64 cross features, own-norm, partner-const, B-term, pad
        nrm = np.minimum((mat * mat).sum(1), NORM_CLAMP * 2)
        f = np.zeros((68, mat.shape[0]), np.float64)
        f[:m] = s_cross * mat.T
        return f, nrm

    fx, xnrm = feats(x)
    fx[m] = -s_norm * np.minimum(xnrm, NORM_CLAMP)
    fx[m + 1] = 8.0
    fx[m + 2] = 1.0

    fc, cnrm = feats(c)
    fc[m] = 8.0
    fc[m + 1] = -s_norm * np.minimum(cnrm, NORM_CLAMP)
    fc[m + 2] = B8

    # fp8 slices: feature 34s + r -> [34, ..., slice s, ...]
    fx8 = fx.astype(NPF8)
    fc8 = fc.astype(NPF8)

    n_tiles = n // 128
    # cT [34, n_tiles, 2, 128]
    cT = np.ascontiguousarray(
        fc8.reshape(2, 34, n_tiles, 128).transpose(1, 2, 0, 3))
    # per-core xT [34, n_blocks, 2, NT]
    xT_full = fx8.reshape(2, 34, N_CORES, n_blocks, NT)

    # wq [128, 288]: col j (j < 32) = w tile 2j, col 128+j = w tile 2j+1
    ws = (w64 * W_SCALE).astype(NPF8).reshape(n_tiles, 128)
    wq = np.zeros((128, 288), NPF8)
    n_pairs = n_tiles // 2
    wq[:, :n_pairs] = ws[0::2].T
    wq[:, 128:128 + n_pairs] = ws[1::2].T

    in_maps = [
        {
            "xT": np.ascontiguousarray(
                xT_full[:, :, cid].transpose(1, 2, 0, 3)),
            "cT": cT,
            "wq": wq,
        }
        for cid in range(N_CORES)
    ]
    build_args = dict(ks=ks, n=n, b_half=float(b64[0]) / 2.0)
    return build_args, in_maps


def kernel(x, x_basis, w, b):
    global LAST_RESULT
    build_args, in_maps = host_setup(x, x_basis, w, b)
    nc = bacc.Bacc("TRN2", target_bir_lowering=False, debug=False,
                   num_devices=N_CORES)
    _build(nc, **build_args)
    nc.compile()
    r = run_bass_kernel_spmd(
        nc, in_maps, list(range(N_CORES)),
        trace=bool(os.environ.get("BASS_KERNEL_TRACE")))
    LAST_RESULT = r
    return np.concatenate([r.results[i]["out"] for i in range(N_CORES)], 0)


# revision 14
# speedup vs baseline: 1.2258x; 1.0310x over previous
"""Trainium2 Bass kernel for LogisticRegressionRBF.

Computes sigmoid(exp(-||x_i - c_j||^2) @ w + b) for x [K, M], c [N, M],
w [N], b [1] with K = N = 8192, M = 64, sharded data-parallel over rows
of x across 8 NeuronCores (KS = 1024 rows per core).

v2 architecture — transposed tiles + fp8 DoubleRow matmuls:
  - All feature matrices are fp8e4m3, with the e4m3 Schraudolph constants
    folded in: the PE produces P[n, x] = A8*R + B8 in PSUM where R =
    -||x - c||^2, A8 = 8*log2(e), B8 = 56 — so int8(max(P, 0)) IS the
    e4m3 bit pattern of exp(R). The 68-dim augmented contraction is split
    into 2 slices of 34 and both main and reduce matmuls run in fp8
    DoubleRow mode (0.5 PE cycles/row, 2x bf16 throughput).
  - Layout is transposed vs v1: each PSUM tile is phi^T [128 n-rows, 512
    x-cols], so the weighted n-reduction is itself a (DoubleRow) matmul:
    lhsT = a [128, 2, 128] view of a packed w table (col 0 = the real
    w pair, cols 1..127 garbage that lands in never-read PSUM partitions
    — the dual-fp8 ldweights ISA check requires 2 contiguous 128-col
    slices), rhs = the exp'd phi pair, accumulated into one PSUM bank
    per x-block across all 32 n-tile pairs.  No sign-sorting, no ln|w|
    folding, no on-engine reductions at all.
  - The exp itself is the only per-element work: PSUM pair-chunks
    [128, 1024] go to ACT (Exp activation, fp8 out) or DVE (Schraudolph
    tensor_scalar_max f32 -> int8, bitcast fp8), strictly alternating so
    both engine pipelines stay decoupled on the 3-deep PSUM chunk ring
    (ACT and DVE are the only engines that can read PSUM — GPSIMD and
    DMA are rejected by the BIR verifier — so they bound the kernel at
    ~38us; the PE runs at only ~43% occupancy).
  - DR-reduces are emitted 2 chunks late so they are always dep-satisfied
    at dispatch and never head-block main matmuls in the in-order PE
    queue (this ordering is worth ~1.3x by itself).
  - Tail per x-block: sigmoid(z) = 0.5*tanh((z/256 + b)/2) + 0.5 on the
    [1, 512] z row; block tails are emitted 6 chunks into the NEXT block
    so the tanh never stalls ACT's exp stream at the block transition.
"""

import os
import sys
from contextlib import ExitStack

import numpy as np

try:
    import concourse.bass as bass  # noqa: F401
except ImportError:  # fresh grading dir: framework lives on these paths
    for _p in (
        "/root/.axon_site/_ro/trn_rl_repo",
        "/root/.axon_site/_ro/pypackages",
        "/opt/trn_rl_repo",
        "/opt/pypackages",
    ):
        if os.path.isdir(_p) and _p not in sys.path:
            sys.path.append(_p)
    import concourse.bass as bass  # noqa: F401

import ml_dtypes
import concourse.tile as tile
from concourse import bacc, mybir
from concourse.bass_utils import run_bass_kernel_spmd

F32 = mybir.dt.float32
F8 = mybir.dt.float8e4
I8 = mybir.dt.int8
AF = mybir.ActivationFunctionType
PM = mybir.MatmulPerfMode
NPF8 = ml_dtypes.float8_e4m3

N_CORES = 8
NT = 512            # x-block width (PSUM bank / matmul moving free dim)
PHI_BUFS = 8        # phi pair-tile buffering depth
W_SCALE = 256.0     # w prescale so fp8 w doesn't underflow (undone in tail)

# e4m3 Schraudolph: bits(exp(R)) = A8*R + B8 for R <= 0
A8 = 8.0 * 1.4426950408889634   # 8*log2(e)
B8 = 56.0                        # 7 (bias) * 8
EXP_SCALE = float(np.float32(1.0 / A8))       # ACT path: exp(P*s + c)
EXP_BIAS = float(np.float32(-B8 / A8))
NORM_CLAMP = 150.0  # host clamp on ||.||^2 so A8/8*norm stays in fp8 range

# chunk engine assignment, chunk = pair = [128, 1024] f32 in PSUM:
#   ACT: Exp activation straight from PSUM -> fp8 SBUF
#   DVE: tensor_scalar_max f32 -> int8 straight from PSUM
#   POOL: DMA stages the PSUM chunk to SBUF f32, then GPSIMD does the
#         Schraudolph max from SBUF (GPSIMD cannot touch PSUM); the DMA
#         engines are otherwise idle, so this is a free third drain
# greedy-balanced statically below using per-chunk cost estimates (ns)
ACT_COST = 1192.0
DVE_COST = 1192.0

LAST_RESULT = None  # BassKernelResults of the most recent run (for test.py)


def _chunk_plan(n_chunks_total):
    """Greedy A/D balance (31A/33D, mostly alternating) — measured best:
    strict-er patterns and ACT-heavier splits both sim slower; the split
    must fold in ACT's fixed work (act-table load, warm-up, tanh tails)."""
    busy = {"A": 0.0, "D": 0.0}
    cost = {"A": ACT_COST, "D": DVE_COST}
    assign = []
    for _ in range(n_chunks_total):
        k = min(busy, key=lambda e: busy[e] + cost[e])
        assign.append(k)
        busy[k] += cost[k]
    return assign


def _build(nc, ks: int, n: int, b_half: float):
    n_tiles = n // 128          # 64
    n_pairs = n_tiles // 2      # 32
    n_blocks = ks // NT         # 2
    assign = _chunk_plan(n_pairs * n_blocks)

    xT = nc.dram_tensor("xT", [34, n_blocks, 2, NT], F8,
                        kind="ExternalInput").ap()
    cT = nc.dram_tensor("cT", [34, n_tiles, 2, 128], F8,
                        kind="ExternalInput").ap()
    wq = nc.dram_tensor("wq", [128, 288], F8, kind="ExternalInput").ap()
    out = nc.dram_tensor("out", [ks, 1], F32, kind="ExternalOutput").ap()

    with tile.TileContext(nc) as tc, ExitStack() as ctx:
        consts = ctx.enter_context(tc.tile_pool(name="consts", bufs=1))
        psum_pool = ctx.enter_context(
            tc.tile_pool(name="psum", bufs=3, space="PSUM"))
        zpool = ctx.enter_context(
            tc.tile_pool(name="zpool", bufs=2, space="PSUM"))

        # warm-up: f32 1x1 matmul at t~0 pins pe_busy_start to ~0 so all
        # post-3us matmuls run at the full 2.4 GHz p-state; an early Exp
        # activation eats the ACT table load during the DMA lead-in
        warm = consts.tile([128, 1], F32, tag="warm")
        nc.vector.memset(warm[:], EXP_BIAS)
        wps = zpool.tile([128, NT], F32, tag="zb")
        nc.tensor.matmul(wps[:1, :1], warm[:], warm[:], start=True, stop=True)
        warm8 = consts.tile([128, 1], F8, tag="warm8")
        nc.scalar.activation(warm8[:], warm[:], AF.Exp, scale=1.0)

        # DMA lead-in: xT + the first cT tiles gate the first matmul;
        # later cT pieces land well before their chunks drain
        xT_sb = consts.tile([34, n_blocks, 2, NT], F8, tag="xT_sb")
        nc.sync.dma_start(xT_sb[:], xT[:])
        cT_sb = consts.tile([34, n_tiles, 2, 128], F8, tag="cT_sb")
        nc.sync.dma_start(cT_sb[:, :8], cT[:, :8])
        wq_sb = consts.tile([128, 288], F8, tag="wq_sb")
        nc.sync.dma_start(wq_sb[:], wq[:])
        nc.sync.dma_start(cT_sb[:, 8:24], cT[:, 8:24])
        nc.sync.dma_start(cT_sb[:, 24:], cT[:, 24:])

        ebias = consts.tile([128, 1], F32, tag="ebias")
        nc.vector.memset(ebias[:], EXP_BIAS)
        bhalf = consts.tile([1, 1], F32, tag="bhalf")
        nc.vector.memset(bhalf[:], b_half)

        phi_pool = ctx.enter_context(
            tc.tile_pool(name="phi_pool", bufs=PHI_BUFS))

        th = consts.tile([1, ks], F32, tag="th")
        res = consts.tile([1, ks], F32, tag="res")
        out_view = out.rearrange("(a b) c -> b (a c)", b=1)

        wq_base = wq_sb[:]
        wq_pdim = list(wq_base.ap)[0]

        def w_pair_view(j):
            # [[p,128],[128,2],[1,128]] at offset j: slice i col 0 reads
            # wq[p, j + 128 i] = w tile (2j + i); cols 1.. read garbage that
            # lands in never-read PSUM partitions 1..127
            return bass.AP(
                tensor=wq_base.tensor,
                offset=wq_base.offset + j,
                ap=[list(wq_pdim), [128, 2], [1, 128]],
            )

        zb = [zpool.tile([128, NT], F32, tag="zb", name=f"zb{i}")
              for i in range(n_blocks)]

        def emit_reduce(blk, j, phi_t):
            nc.tensor.matmul(
                zb[blk][:],
                w_pair_view(j),
                phi_t[:].rearrange("p (a b) -> p a b", a=2),
                start=(j == 0), stop=(j == n_pairs - 1),
                perf_mode=PM.DoubleRow)

        def emit_tail(blk):
            # z = row 0 of zb; sigmoid(z/W_SCALE + b) via same-table tanh
            o0 = blk * NT
            nc.scalar.activation(th[:, o0:o0 + NT], zb[blk][0:1, :],
                                 AF.Tanh, scale=float(np.float32(
                                     1.0 / (2.0 * W_SCALE))),
                                 bias=bhalf[:])
            nc.vector.tensor_scalar(res[:, o0:o0 + NT], th[:, o0:o0 + NT],
                                    0.5, 0.5, mybir.AluOpType.mult,
                                    mybir.AluOpType.add)
            nc.sync.dma_start(out_view[:, o0:o0 + NT], res[:, o0:o0 + NT])

        ci = 0
        for blk in range(n_blocks):
            xv = xT_sb[:, blk]          # [34, 2, NT]
            phi_tiles = {}
            for c in range(n_pairs):    # chunk c == n-tile pair c
                if blk > 0 and c == 6:
                    # previous block's tail, emitted after this block's
                    # pipeline has refilled so the tanh (queued on ACT)
                    # never stalls the exp stream at the block transition
                    emit_tail(blk - 1)
                # reduces are delayed 2 chunks so they are dep-satisfied at
                # dispatch and never head-block the in-order PE queue
                if c >= 2:
                    emit_reduce(blk, c - 2, phi_tiles.pop(c - 2))
                ps = psum_pool.tile([128, 2 * NT], F32, tag="ps")
                for i in range(2):
                    nc.tensor.matmul(
                        ps[:, i * NT:(i + 1) * NT],
                        cT_sb[:, 2 * c + i],    # [34, 2, 128]
                        xv,
                        start=True, stop=True,
                        perf_mode=PM.DoubleRow)
                phi_t = phi_pool.tile([128, 2 * NT], F8, tag="phi",
                                      name=f"phi_{blk}_{c}")
                phi_tiles[c] = phi_t
                if assign[ci] == "A":
                    nc.scalar.activation(
                        phi_t[:], ps[:], AF.Exp,
                        scale=EXP_SCALE, bias=ebias[:])
                else:
                    nc.vector.tensor_scalar_max(
                        phi_t[:].bitcast(I8), ps[:], 0.0)
                ci += 1
            for c in (n_pairs - 2, n_pairs - 1):
                emit_reduce(blk, c, phi_tiles.pop(c))
        emit_tail(n_blocks - 1)


def host_setup(x, x_basis, w, b):
    """Host-side fp8 feature construction. Returns (build_args, in_maps)."""
    x = np.asarray(x, np.float64)
    c = np.asarray(x_basis, np.float64)
    w64 = np.asarray(w, np.float64)
    b64 = np.asarray(b, np.float64)
    k, m = x.shape
    n = c.shape[0]
    ks = k // N_CORES
    n_blocks = ks // NT

    s_cross = np.sqrt(2.0 * A8)
    s_norm = A8 / 8.0

    def feats(mat):
        # [68, rows]: 64 cross features, own-norm, partner-const, B-term, pad
        nrm = np.minimum((mat * mat).sum(1), NORM_CLAMP * 2)
        f = np.zeros((68, mat.shape[0]), np.float64)
        f[:m] = s_cross * mat.T
        return f, nrm

    fx, xnrm = feats(x)
    fx[m] = -s_norm * np.minimum(xnrm, NORM_CLAMP)
    fx[m + 1] = 8.0
    fx[m + 2] = 1.0

    fc, cnrm = feats(c)
    fc[m] = 8.0
    fc[m + 1] = -s_norm * np.minimum(cnrm, NORM_CLAMP)
    fc[m + 2] = B8

    # fp8 slices: feature 34s + r -> [34, ..., slice s, ...]
    fx8 = fx.astype(NPF8)
    fc8 = fc.astype(NPF8)

    n_tiles = n // 128
    # cT [34, n_tiles, 2, 128]
    cT = np.ascontiguousarray(
        fc8.reshape(2, 34, n_tiles, 128).transpose(1, 2, 0, 3))
    # per-core xT [34, n_blocks, 2, NT]
    xT_full = fx8.reshape(2, 34, N_CORES, n_blocks, NT)

    # wq [128, 288]: col j (j < 32) = w tile 2j, col 128+j = w tile 2j+1
    ws = (w64 * W_SCALE).astype(NPF8).reshape(n_tiles, 128)
    wq = np.zeros((128, 288), NPF8)
    n_pairs = n_tiles // 2
    wq[:, :n_pairs] = ws[0::2].T
    wq[:, 128:128 + n_pairs] = ws[1::2].T

    in_maps = [
        {
            "xT": np.ascontiguousarray(
                xT_full[:, :, cid].transpose(1, 2, 0, 3)),
            "cT": cT,
            "wq": wq,
        }
        for cid in range(N_CORES)
    ]
    build_args = dict(ks=ks, n=n, b_half=float(b64[0]) / 2.0)
    return build_args, in_maps


def kernel(x, x_basis, w, b):
    global LAST_RESULT
    build_args, in_maps = host_setup(x, x_basis, w, b)
    nc = bacc.Bacc("TRN2", target_bir_lowering=False, debug=False,
                   num_devices=N_CORES)
    _build(nc, **build_args)
    nc.compile()
    r = run_bass_kernel_spmd(
        nc, in_maps, list(range(N_CORES)),
        trace=bool(os.environ.get("BASS_KERNEL_TRACE")))
    LAST_RESULT = r
    return np.concatenate([r.results[i]["out"] for i in range(N_CORES)], 0)


# revision 15
# speedup vs baseline: 1.2309x; 1.0041x over previous
"""Trainium2 Bass kernel for LogisticRegressionRBF.

Computes sigmoid(exp(-||x_i - c_j||^2) @ w + b) for x [K, M], c [N, M],
w [N], b [1] with K = N = 8192, M = 64, sharded data-parallel over rows
of x across 8 NeuronCores (KS = 1024 rows per core).

v2 architecture — transposed tiles + fp8 DoubleRow matmuls:
  - All feature matrices are fp8e4m3, with the e4m3 Schraudolph constants
    folded in: the PE produces P[n, x] = A8*R + B8 in PSUM where R =
    -||x - c||^2, A8 = 8*log2(e), B8 = 56 — so int8(max(P, 0)) IS the
    e4m3 bit pattern of exp(R). The 68-dim augmented contraction is split
    into 2 slices of 34 and both main and reduce matmuls run in fp8
    DoubleRow mode (0.5 PE cycles/row, 2x bf16 throughput).
  - Layout is transposed vs v1: each PSUM tile is phi^T [128 n-rows, 512
    x-cols], so the weighted n-reduction is itself a (DoubleRow) matmul:
    lhsT = a [128, 2, 128] view of a packed w table (col 0 = the real
    w pair, cols 1..127 garbage that lands in never-read PSUM partitions
    — the dual-fp8 ldweights ISA check requires 2 contiguous 128-col
    slices), rhs = the exp'd phi pair, accumulated into one PSUM bank
    per x-block across all 32 n-tile pairs.  No sign-sorting, no ln|w|
    folding, no on-engine reductions at all.
  - The exp itself is the only per-element work: PSUM pair-chunks
    [128, 1024] go to ACT (Exp activation, fp8 out) or DVE (Schraudolph
    tensor_scalar_max f32 -> int8, bitcast fp8), strictly alternating so
    both engine pipelines stay decoupled on the 3-deep PSUM chunk ring
    (ACT and DVE are the only engines that can read PSUM — GPSIMD and
    DMA are rejected by the BIR verifier — so they bound the kernel at
    ~38us; the PE runs at only ~43% occupancy).
  - DR-reduces are emitted 2 chunks late so they are always dep-satisfied
    at dispatch and never head-block main matmuls in the in-order PE
    queue (this ordering is worth ~1.3x by itself).
  - Tail per x-block: sigmoid(z) = 0.5*tanh((z/256 + b)/2) + 0.5 on the
    [1, 512] z row; block tails are emitted 6 chunks into the NEXT block
    so the tanh never stalls ACT's exp stream at the block transition.
"""

import os
import sys
from contextlib import ExitStack

import numpy as np

try:
    import concourse.bass as bass  # noqa: F401
except ImportError:  # fresh grading dir: framework lives on these paths
    for _p in (
        "/root/.axon_site/_ro/trn_rl_repo",
        "/root/.axon_site/_ro/pypackages",
        "/opt/trn_rl_repo",
        "/opt/pypackages",
    ):
        if os.path.isdir(_p) and _p not in sys.path:
            sys.path.append(_p)
    import concourse.bass as bass  # noqa: F401

import ml_dtypes
import concourse.tile as tile
from concourse import bacc, mybir
from concourse.bass_utils import run_bass_kernel_spmd

F32 = mybir.dt.float32
F8 = mybir.dt.float8e4
I8 = mybir.dt.int8
AF = mybir.ActivationFunctionType
PM = mybir.MatmulPerfMode
NPF8 = ml_dtypes.float8_e4m3

N_CORES = 8
NT = 512            # x-block width (PSUM bank / matmul moving free dim)
PHI_BUFS = 8        # phi pair-tile buffering depth
W_SCALE = 256.0     # w prescale so fp8 w doesn't underflow (undone in tail)

# e4m3 Schraudolph: bits(exp(R)) = A8*R + B8 for R <= 0
A8 = 8.0 * 1.4426950408889634   # 8*log2(e)
B8 = 56.0                        # 7 (bias) * 8
EXP_SCALE = float(np.float32(1.0 / A8))       # ACT path: exp(P*s + c)
EXP_BIAS = float(np.float32(-B8 / A8))
NORM_CLAMP = 150.0  # host clamp on ||.||^2 so A8/8*norm stays in fp8 range

# chunk engine assignment, chunk = pair = [128, 1024] f32 in PSUM:
#   ACT: Exp activation straight from PSUM -> fp8 SBUF
#   DVE: tensor_scalar_max f32 -> int8 straight from PSUM
#   POOL: DMA stages the PSUM chunk to SBUF f32, then GPSIMD does the
#         Schraudolph max from SBUF (GPSIMD cannot touch PSUM); the DMA
#         engines are otherwise idle, so this is a free third drain
# greedy-balanced statically below using per-chunk cost estimates (ns)
ACT_COST = 1192.0
DVE_COST = 1192.0

LAST_RESULT = None  # BassKernelResults of the most recent run (for test.py)


def _chunk_plan(n_chunks_total):
    """Greedy A/D balance (31A/33D, mostly alternating) — measured best:
    strict-er patterns and ACT-heavier splits both sim slower; the split
    must fold in ACT's fixed work (act-table load, warm-up, tanh tails)."""
    busy = {"A": 0.0, "D": 0.0}
    cost = {"A": ACT_COST, "D": DVE_COST}
    assign = []
    for _ in range(n_chunks_total):
        k = min(busy, key=lambda e: busy[e] + cost[e])
        assign.append(k)
        busy[k] += cost[k]
    return assign


def _build(nc, ks: int, n: int, b_half: float):
    n_tiles = n // 128          # 64
    n_pairs = n_tiles // 2      # 32
    n_blocks = ks // NT         # 2
    assign = _chunk_plan(n_pairs * n_blocks)

    xT = nc.dram_tensor("xT", [34, n_blocks, 2, NT], F8,
                        kind="ExternalInput").ap()
    cT = nc.dram_tensor("cT", [34, n_tiles, 2, 128], F8,
                        kind="ExternalInput").ap()
    wq = nc.dram_tensor("wq", [128, 288], F8, kind="ExternalInput").ap()
    out = nc.dram_tensor("out", [ks, 1], F32, kind="ExternalOutput").ap()

    with tile.TileContext(nc) as tc, ExitStack() as ctx:
        consts = ctx.enter_context(tc.tile_pool(name="consts", bufs=1))
        psum_pool = ctx.enter_context(
            tc.tile_pool(name="psum", bufs=3, space="PSUM"))
        zpool = ctx.enter_context(
            tc.tile_pool(name="zpool", bufs=2, space="PSUM"))

        # warm-up: f32 1x1 matmul at t~0 pins pe_busy_start to ~0 so all
        # post-3us matmuls run at the full 2.4 GHz p-state; an early Exp
        # activation eats the ACT table load during the DMA lead-in
        warm = consts.tile([128, 1], F32, tag="warm")
        nc.vector.memset(warm[:], EXP_BIAS)
        wps = zpool.tile([128, NT], F32, tag="zb")
        nc.tensor.matmul(wps[:1, :1], warm[:], warm[:], start=True, stop=True)
        warm8 = consts.tile([128, 1], F8, tag="warm8")
        nc.scalar.activation(warm8[:], warm[:], AF.Exp, scale=1.0)

        # DMA lead-in: xT + the first cT tiles gate the first matmul;
        # later cT pieces land well before their chunks drain
        xT_sb = consts.tile([34, n_blocks, 2, NT], F8, tag="xT_sb")
        nc.sync.dma_start(xT_sb[:], xT[:])
        cT_sb = consts.tile([34, n_tiles, 2, 128], F8, tag="cT_sb")
        nc.sync.dma_start(cT_sb[:, :8], cT[:, :8])
        wq_sb = consts.tile([128, 288], F8, tag="wq_sb")
        nc.sync.dma_start(wq_sb[:], wq[:])
        nc.sync.dma_start(cT_sb[:, 8:24], cT[:, 8:24])
        nc.sync.dma_start(cT_sb[:, 24:], cT[:, 24:])

        ebias = consts.tile([128, 1], F32, tag="ebias")
        nc.vector.memset(ebias[:], EXP_BIAS)
        bhalf = consts.tile([1, 1], F32, tag="bhalf")
        nc.vector.memset(bhalf[:], b_half)

        phi_pool = ctx.enter_context(
            tc.tile_pool(name="phi_pool", bufs=PHI_BUFS))

        th = consts.tile([1, ks], F32, tag="th")
        res = consts.tile([1, ks], F32, tag="res")
        out_view = out.rearrange("(a b) c -> b (a c)", b=1)

        wq_base = wq_sb[:]
        wq_pdim = list(wq_base.ap)[0]

        def w_pair_view(j):
            # [[p,128],[128,2],[1,128]] at offset j: slice i col 0 reads
            # wq[p, j + 128 i] = w tile (2j + i); cols 1.. read garbage that
            # lands in never-read PSUM partitions 1..127
            return bass.AP(
                tensor=wq_base.tensor,
                offset=wq_base.offset + j,
                ap=[list(wq_pdim), [128, 2], [1, 128]],
            )

        zb = [zpool.tile([128, NT], F32, tag="zb", name=f"zb{i}")
              for i in range(n_blocks)]

        def emit_reduce(blk, j, phi_t):
            nc.tensor.matmul(
                zb[blk][:],
                w_pair_view(j),
                phi_t[:].rearrange("p (a b) -> p a b", a=2),
                start=(j == 0), stop=(j == n_pairs - 1),
                perf_mode=PM.DoubleRow)

        def emit_tail(blk):
            # z = row 0 of zb; sigmoid(z/W_SCALE + b) via same-table tanh
            o0 = blk * NT
            nc.scalar.activation(th[:, o0:o0 + NT], zb[blk][0:1, :],
                                 AF.Tanh, scale=float(np.float32(
                                     1.0 / (2.0 * W_SCALE))),
                                 bias=bhalf[:])
            nc.vector.tensor_scalar(res[:, o0:o0 + NT], th[:, o0:o0 + NT],
                                    0.5, 0.5, mybir.AluOpType.mult,
                                    mybir.AluOpType.add)
            nc.sync.dma_start(out_view[:, o0:o0 + NT], res[:, o0:o0 + NT])

        ci = 0
        for blk in range(n_blocks):
            xv = xT_sb[:, blk]          # [34, 2, NT]
            phi_tiles = {}
            for c in range(n_pairs):    # chunk c == n-tile pair c
                if blk > 0 and c == 6:
                    # previous block's tail, emitted after this block's
                    # pipeline has refilled so the tanh (queued on ACT)
                    # never stalls the exp stream at the block transition
                    emit_tail(blk - 1)
                # reduces are delayed 2 chunks so they are dep-satisfied at
                # dispatch and never head-block the in-order PE queue
                if c >= 3:
                    emit_reduce(blk, c - 3, phi_tiles.pop(c - 3))
                ps = psum_pool.tile([128, 2 * NT], F32, tag="ps")
                for i in range(2):
                    nc.tensor.matmul(
                        ps[:, i * NT:(i + 1) * NT],
                        cT_sb[:, 2 * c + i],    # [34, 2, 128]
                        xv,
                        start=True, stop=True,
                        perf_mode=PM.DoubleRow)
                phi_t = phi_pool.tile([128, 2 * NT], F8, tag="phi",
                                      name=f"phi_{blk}_{c}")
                phi_tiles[c] = phi_t
                if assign[ci] == "A":
                    nc.scalar.activation(
                        phi_t[:], ps[:], AF.Exp,
                        scale=EXP_SCALE, bias=ebias[:])
                else:
                    nc.vector.tensor_scalar_max(
                        phi_t[:].bitcast(I8), ps[:], 0.0)
                ci += 1
            for c in (n_pairs - 3, n_pairs - 2, n_pairs - 1):
                emit_reduce(blk, c, phi_tiles.pop(c))
        emit_tail(n_blocks - 1)


def host_setup(x, x_basis, w, b):
    """Host-side fp8 feature construction. Returns (build_args, in_maps)."""
    x = np.asarray(x, np.float64)
    c = np.asarray(x_basis, np.float64)
    w64 = np.asarray(w, np.float64)
    b64 = np.asarray(b, np.float64)
    k, m = x.shape
    n = c.shape[0]
    ks = k // N_CORES
    n_blocks = ks // NT

    s_cross = np.sqrt(2.0 * A8)
    s_norm = A8 / 8.0

    def feats(mat):
        # [68, rows]: 64 cross features, own-norm, partner-const, B-term, pad
        nrm = np.minimum((mat * mat).sum(1), NORM_CLAMP * 2)
        f = np.zeros((68, mat.shape[0]), np.float64)
        f[:m] = s_cross * mat.T
        return f, nrm

    fx, xnrm = feats(x)
    fx[m] = -s_norm * np.minimum(xnrm, NORM_CLAMP)
    fx[m + 1] = 8.0
    fx[m + 2] = 1.0

    fc, cnrm = feats(c)
    fc[m] = 8.0
    fc[m + 1] = -s_norm * np.minimum(cnrm, NORM_CLAMP)
    fc[m + 2] = B8

    # fp8 slices: feature 34s + r -> [34, ..., slice s, ...]
    fx8 = fx.astype(NPF8)
    fc8 = fc.astype(NPF8)

    n_tiles = n // 128
    # cT [34, n_tiles, 2, 128]
    cT = np.ascontiguousarray(
        fc8.reshape(2, 34, n_tiles, 128).transpose(1, 2, 0, 3))
    # per-core xT [34, n_blocks, 2, NT]
    xT_full = fx8.reshape(2, 34, N_CORES, n_blocks, NT)

    # wq [128, 288]: col j (j < 32) = w tile 2j, col 128+j = w tile 2j+1
    ws = (w64 * W_SCALE).astype(NPF8).reshape(n_tiles, 128)
    wq = np.zeros((128, 288), NPF8)
    n_pairs = n_tiles // 2
    wq[:, :n_pairs] = ws[0::2].T
    wq[:, 128:128 + n_pairs] = ws[1::2].T

    in_maps = [
        {
            "xT": np.ascontiguousarray(
                xT_full[:, :, cid].transpose(1, 2, 0, 3)),
            "cT": cT,
            "wq": wq,
        }
        for cid in range(N_CORES)
    ]
    build_args = dict(ks=ks, n=n, b_half=float(b64[0]) / 2.0)
    return build_args, in_maps


def kernel(x, x_basis, w, b):
    global LAST_RESULT
    build_args, in_maps = host_setup(x, x_basis, w, b)
    nc = bacc.Bacc("TRN2", target_bir_lowering=False, debug=False,
                   num_devices=N_CORES)
    _build(nc, **build_args)
    nc.compile()
    r = run_bass_kernel_spmd(
        nc, in_maps, list(range(N_CORES)),
        trace=bool(os.environ.get("BASS_KERNEL_TRACE")))
    LAST_RESULT = r
    return np.concatenate([r.results[i]["out"] for i in range(N_CORES)], 0)


# revision 16
# speedup vs baseline: 1.2375x; 1.0054x over previous
"""Trainium2 Bass kernel for LogisticRegressionRBF.

Computes sigmoid(exp(-||x_i - c_j||^2) @ w + b) for x [K, M], c [N, M],
w [N], b [1] with K = N = 8192, M = 64, sharded data-parallel over rows
of x across 8 NeuronCores (KS = 1024 rows per core).

v2 architecture — transposed tiles + fp8 DoubleRow matmuls:
  - All feature matrices are fp8e4m3, with the e4m3 Schraudolph constants
    folded in: the PE produces P[n, x] = A8*R + B8 in PSUM where R =
    -||x - c||^2, A8 = 8*log2(e), B8 = 56 — so int8(max(P, 0)) IS the
    e4m3 bit pattern of exp(R). The 68-dim augmented contraction is split
    into 2 slices of 34 and both main and reduce matmuls run in fp8
    DoubleRow mode (0.5 PE cycles/row, 2x bf16 throughput).
  - Layout is transposed vs v1: each PSUM tile is phi^T [128 n-rows, 512
    x-cols], so the weighted n-reduction is itself a (DoubleRow) matmul:
    lhsT = a [128, 2, 128] view of a packed w table (col 0 = the real
    w pair, cols 1..127 garbage that lands in never-read PSUM partitions
    — the dual-fp8 ldweights ISA check requires 2 contiguous 128-col
    slices), rhs = the exp'd phi pair, accumulated into one PSUM bank
    per x-block across all 32 n-tile pairs.  No sign-sorting, no ln|w|
    folding, no on-engine reductions at all.
  - The exp itself is the only per-element work: PSUM pair-chunks
    [128, 1024] go to ACT (Exp activation, fp8 out) or DVE (Schraudolph
    tensor_scalar_max f32 -> int8, bitcast fp8), strictly alternating so
    both engine pipelines stay decoupled on the 3-deep PSUM chunk ring
    (ACT and DVE are the only engines that can read PSUM — GPSIMD and
    DMA are rejected by the BIR verifier — so they bound the kernel at
    ~38us; the PE runs at only ~43% occupancy).
  - DR-reduces are emitted 4 chunks late so they are always dep-satisfied
    at dispatch and never head-block main matmuls in the in-order PE
    queue (this ordering is worth ~1.3x by itself; delay 4 sims best).
  - Tail per x-block: sigmoid(z) = 0.5*tanh((z/256 + b)/2) + 0.5 on the
    [1, 512] z row; block tails are emitted 6 chunks into the NEXT block
    so the tanh never stalls ACT's exp stream at the block transition.
"""

import os
import sys
from contextlib import ExitStack

import numpy as np

try:
    import concourse.bass as bass  # noqa: F401
except ImportError:  # fresh grading dir: framework lives on these paths
    for _p in (
        "/root/.axon_site/_ro/trn_rl_repo",
        "/root/.axon_site/_ro/pypackages",
        "/opt/trn_rl_repo",
        "/opt/pypackages",
    ):
        if os.path.isdir(_p) and _p not in sys.path:
            sys.path.append(_p)
    import concourse.bass as bass  # noqa: F401

import ml_dtypes
import concourse.tile as tile
from concourse import bacc, mybir
from concourse.bass_utils import run_bass_kernel_spmd

F32 = mybir.dt.float32
F8 = mybir.dt.float8e4
I8 = mybir.dt.int8
AF = mybir.ActivationFunctionType
PM = mybir.MatmulPerfMode
NPF8 = ml_dtypes.float8_e4m3

N_CORES = 8
NT = 512            # x-block width (PSUM bank / matmul moving free dim)
PHI_BUFS = 8        # phi pair-tile buffering depth
W_SCALE = 256.0     # w prescale so fp8 w doesn't underflow (undone in tail)

# e4m3 Schraudolph: bits(exp(R)) = A8*R + B8 for R <= 0
A8 = 8.0 * 1.4426950408889634   # 8*log2(e)
B8 = 56.0                        # 7 (bias) * 8
EXP_SCALE = float(np.float32(1.0 / A8))       # ACT path: exp(P*s + c)
EXP_BIAS = float(np.float32(-B8 / A8))
NORM_CLAMP = 150.0  # host clamp on ||.||^2 so A8/8*norm stays in fp8 range

# chunk engine assignment, chunk = pair = [128, 1024] f32 in PSUM:
#   ACT: Exp activation straight from PSUM -> fp8 SBUF
#   DVE: tensor_scalar_max f32 -> int8 straight from PSUM
# (GPSIMD and DMA cannot touch PSUM — BIR verifier — so ACT+DVE are the
# only possible drains and bound the kernel)
ACT_COST = 1192.0
DVE_COST = 1192.0

LAST_RESULT = None  # BassKernelResults of the most recent run (for test.py)


def _chunk_plan(n_chunks_total):
    """Strict A/D alternation (32/32) — measured best: ACT-heavier splits
    sim slower because same-engine runs stall the other engine on the
    shared 3-deep PSUM chunk ring."""
    busy = {"A": 0.0, "D": 0.0}
    cost = {"A": ACT_COST, "D": DVE_COST}
    assign = []
    for _ in range(n_chunks_total):
        k = min(busy, key=lambda e: busy[e] + cost[e])
        assign.append(k)
        busy[k] += cost[k]
    return assign


def _build(nc, ks: int, n: int, b_half: float):
    n_tiles = n // 128          # 64
    n_pairs = n_tiles // 2      # 32
    n_blocks = ks // NT         # 2
    assign = _chunk_plan(n_pairs * n_blocks)

    xT = nc.dram_tensor("xT", [34, n_blocks, 2, NT], F8,
                        kind="ExternalInput").ap()
    cT = nc.dram_tensor("cT", [34, n_tiles, 2, 128], F8,
                        kind="ExternalInput").ap()
    wq = nc.dram_tensor("wq", [128, 288], F8, kind="ExternalInput").ap()
    out = nc.dram_tensor("out", [ks, 1], F32, kind="ExternalOutput").ap()

    with tile.TileContext(nc) as tc, ExitStack() as ctx:
        consts = ctx.enter_context(tc.tile_pool(name="consts", bufs=1))
        psum_pool = ctx.enter_context(
            tc.tile_pool(name="psum", bufs=3, space="PSUM"))
        zpool = ctx.enter_context(
            tc.tile_pool(name="zpool", bufs=2, space="PSUM"))

        # warm-up: f32 1x1 matmul at t~0 pins pe_busy_start to ~0 so all
        # post-3us matmuls run at the full 2.4 GHz p-state; an early Exp
        # activation eats the ACT table load during the DMA lead-in
        warm = consts.tile([128, 1], F32, tag="warm")
        nc.vector.memset(warm[:], EXP_BIAS)
        wps = zpool.tile([128, NT], F32, tag="zb")
        nc.tensor.matmul(wps[:1, :1], warm[:], warm[:], start=True, stop=True)
        warm8 = consts.tile([128, 1], F8, tag="warm8")
        nc.scalar.activation(warm8[:], warm[:], AF.Exp, scale=1.0)

        # DMA lead-in: xT + the first cT tiles gate the first matmul;
        # later cT pieces land well before their chunks drain
        xT_sb = consts.tile([34, n_blocks, 2, NT], F8, tag="xT_sb")
        nc.sync.dma_start(xT_sb[:], xT[:])
        cT_sb = consts.tile([34, n_tiles, 2, 128], F8, tag="cT_sb")
        nc.sync.dma_start(cT_sb[:, :8], cT[:, :8])
        wq_sb = consts.tile([128, 288], F8, tag="wq_sb")
        nc.sync.dma_start(wq_sb[:], wq[:])
        nc.sync.dma_start(cT_sb[:, 8:24], cT[:, 8:24])
        nc.sync.dma_start(cT_sb[:, 24:], cT[:, 24:])

        ebias = consts.tile([128, 1], F32, tag="ebias")
        nc.vector.memset(ebias[:], EXP_BIAS)
        bhalf = consts.tile([1, 1], F32, tag="bhalf")
        nc.vector.memset(bhalf[:], b_half)

        phi_pool = ctx.enter_context(
            tc.tile_pool(name="phi_pool", bufs=PHI_BUFS))

        th = consts.tile([1, ks], F32, tag="th")
        res = consts.tile([1, ks], F32, tag="res")
        out_view = out.rearrange("(a b) c -> b (a c)", b=1)

        wq_base = wq_sb[:]
        wq_pdim = list(wq_base.ap)[0]

        def w_pair_view(j):
            # [[p,128],[128,2],[1,128]] at offset j: slice i col 0 reads
            # wq[p, j + 128 i] = w tile (2j + i); cols 1.. read garbage that
            # lands in never-read PSUM partitions 1..127
            return bass.AP(
                tensor=wq_base.tensor,
                offset=wq_base.offset + j,
                ap=[list(wq_pdim), [128, 2], [1, 128]],
            )

        zb = [zpool.tile([128, NT], F32, tag="zb", name=f"zb{i}")
              for i in range(n_blocks)]

        def emit_reduce(blk, j, phi_t):
            nc.tensor.matmul(
                zb[blk][:],
                w_pair_view(j),
                phi_t[:].rearrange("p (a b) -> p a b", a=2),
                start=(j == 0), stop=(j == n_pairs - 1),
                perf_mode=PM.DoubleRow)

        def emit_tail(blk):
            # z = row 0 of zb; sigmoid(z/W_SCALE + b) via same-table tanh
            o0 = blk * NT
            nc.scalar.activation(th[:, o0:o0 + NT], zb[blk][0:1, :],
                                 AF.Tanh, scale=float(np.float32(
                                     1.0 / (2.0 * W_SCALE))),
                                 bias=bhalf[:])
            nc.vector.tensor_scalar(res[:, o0:o0 + NT], th[:, o0:o0 + NT],
                                    0.5, 0.5, mybir.AluOpType.mult,
                                    mybir.AluOpType.add)
            nc.sync.dma_start(out_view[:, o0:o0 + NT], res[:, o0:o0 + NT])

        ci = 0
        for blk in range(n_blocks):
            xv = xT_sb[:, blk]          # [34, 2, NT]
            phi_tiles = {}
            for c in range(n_pairs):    # chunk c == n-tile pair c
                if blk > 0 and c == 6:
                    # previous block's tail, emitted after this block's
                    # pipeline has refilled so the tanh (queued on ACT)
                    # never stalls the exp stream at the block transition
                    emit_tail(blk - 1)
                # reduces are delayed 2 chunks so they are dep-satisfied at
                # dispatch and never head-block the in-order PE queue
                if c >= 4:
                    emit_reduce(blk, c - 4, phi_tiles.pop(c - 4))
                ps = psum_pool.tile([128, 2 * NT], F32, tag="ps")
                for i in range(2):
                    nc.tensor.matmul(
                        ps[:, i * NT:(i + 1) * NT],
                        cT_sb[:, 2 * c + i],    # [34, 2, 128]
                        xv,
                        start=True, stop=True,
                        perf_mode=PM.DoubleRow)
                phi_t = phi_pool.tile([128, 2 * NT], F8, tag="phi",
                                      name=f"phi_{blk}_{c}")
                phi_tiles[c] = phi_t
                if assign[ci] == "A":
                    nc.scalar.activation(
                        phi_t[:], ps[:], AF.Exp,
                        scale=EXP_SCALE, bias=ebias[:])
                else:
                    nc.vector.tensor_scalar_max(
                        phi_t[:].bitcast(I8), ps[:], 0.0)
                ci += 1
            for c in (n_pairs - 4, n_pairs - 3, n_pairs - 2, n_pairs - 1):
                emit_reduce(blk, c, phi_tiles.pop(c))
        emit_tail(n_blocks - 1)


def host_setup(x, x_basis, w, b):
    """Host-side fp8 feature construction. Returns (build_args, in_maps)."""
    x = np.asarray(x, np.float64)
    c = np.asarray(x_basis, np.float64)
    w64 = np.asarray(w, np.float64)
    b64 = np.asarray(b, np.float64)
    k, m = x.shape
    n = c.shape[0]
    ks = k // N_CORES
    n_blocks = ks // NT

    s_cross = np.sqrt(2.0 * A8)
    s_norm = A8 / 8.0

    def feats(mat):
        # [68, rows]: 64 cross features, own-norm, partner-const, B-term, pad
        nrm = np.minimum((mat * mat).sum(1), NORM_CLAMP * 2)
        f = np.zeros((68, mat.shape[0]), np.float64)
        f[:m] = s_cross * mat.T
        return f, nrm

    fx, xnrm = feats(x)
    fx[m] = -s_norm * np.minimum(xnrm, NORM_CLAMP)
    fx[m + 1] = 8.0
    fx[m + 2] = 1.0

    fc, cnrm = feats(c)
    fc[m] = 8.0
    fc[m + 1] = -s_norm * np.minimum(cnrm, NORM_CLAMP)
    fc[m + 2] = B8

    # fp8 slices: feature 34s + r -> [34, ..., slice s, ...]
    fx8 = fx.astype(NPF8)
    fc8 = fc.astype(NPF8)

    n_tiles = n // 128
    # cT [34, n_tiles, 2, 128]
    cT = np.ascontiguousarray(
        fc8.reshape(2, 34, n_tiles, 128).transpose(1, 2, 0, 3))
    # per-core xT [34, n_blocks, 2, NT]
    xT_full = fx8.reshape(2, 34, N_CORES, n_blocks, NT)

    # wq [128, 288]: col j (j < 32) = w tile 2j, col 128+j = w tile 2j+1
    ws = (w64 * W_SCALE).astype(NPF8).reshape(n_tiles, 128)
    wq = np.zeros((128, 288), NPF8)
    n_pairs = n_tiles // 2
    wq[:, :n_pairs] = ws[0::2].T
    wq[:, 128:128 + n_pairs] = ws[1::2].T

    in_maps = [
        {
            "xT": np.ascontiguousarray(
                xT_full[:, :, cid].transpose(1, 2, 0, 3)),
            "cT": cT,
            "wq": wq,
        }
        for cid in range(N_CORES)
    ]
    build_args = dict(ks=ks, n=n, b_half=float(b64[0]) / 2.0)
    return build_args, in_maps


def kernel(x, x_basis, w, b):
    global LAST_RESULT
    build_args, in_maps = host_setup(x, x_basis, w, b)
    nc = bacc.Bacc("TRN2", target_bir_lowering=False, debug=False,
                   num_devices=N_CORES)
    _build(nc, **build_args)
    nc.compile()
    r = run_bass_kernel_spmd(
        nc, in_maps, list(range(N_CORES)),
        trace=bool(os.environ.get("BASS_KERNEL_TRACE")))
    LAST_RESULT = r
    return np.concatenate([r.results[i]["out"] for i in range(N_CORES)], 0)
